# revision 43
# baseline (speedup 1.0000x reference)
"""AdaLN attention block (DiT-style) on 8 TRN2 NeuronCores.

Sharding: 8 cores = 4 batches x 2 query-token halves, no collectives. Core c
handles batch c//2 and query half c%2: layernorm1 and k/v cover the full
(permuted) sequence; everything else covers only the own 512 query rows.

Device layout is feature-major (activations transposed, [d, n]). X @ W runs
with W column-tiles stationary and X^T moving, producing Y^T directly.
LayerNorm statistics use ones-vector matmuls (partition-axis sums on the PE);
the AdaLN modulate is h = x*A + B with rank-1 A/B built by K=1/K=2
outer-product matmuls into PSUM. Softmax skips max-subtraction (exp in fp32
psum, bf16 e tiles); the denominator is a ones-column appended to the attn@v
stationary operand; per-head normalization uses a fast DVE reciprocal and a
PE row-broadcast.

v2 changes vs the original baseline:
- x shipped bf16 (halves DMA, kills cast ops); bo folded with the v-bias on
  host (bo_eff = bo + bv @ Wo).
- exp runs on 1024-wide psum tensors (half the Act-engine dispatch cost).
- softmax/LN reciprocals use reciprocal_approx_fast/accurate (5x faster).
- elementwise work split across DVE and Pool (gpsimd) engines.
- epilogues fused into single scalar_tensor_tensor ops.
- Wada/mod matmul groups 4-11 are interleaved into the attention loop where
  the PE otherwise waits on exp results.
- weights arrive via merged ~1MB DMAs (one per 512-column group).
"""

import numpy as np
from contextlib import ExitStack

import concourse.bass as bass
import concourse.bacc as bacc
import concourse.mybir as mybir
from concourse import tile
from concourse.bass_utils import run_bass_kernel_spmd

P = 128
D = 1024
N = 1024
NQ = 512
H = 16
DH = 64
MLPD = 4096
EPS = 1e-6
NCORES = 8

F32 = mybir.dt.float32
BF16 = mybir.dt.bfloat16
AF = mybir.ActivationFunctionType
ALU = mybir.AluOpType

KT = D // P           # 8 contraction tiles over D
MT = MLPD // P        # 32 tiles over MLP dim


def build():
    nc = bacc.Bacc("TRN2", target_bir_lowering=False, debug=False,
                   num_devices=NCORES)

    GW = KT * NQ   # 4096 packed columns per 512-wide output group
    xT = nc.dram_tensor("xT", [P, KT * N], BF16, kind="ExternalInput")
    crow = nc.dram_tensor("crow", [1, D], BF16, kind="ExternalInput")
    Wq = nc.dram_tensor("Wq", [P, 2 * GW], BF16, kind="ExternalInput")
    Wkv = nc.dram_tensor("Wkv", [P, 4 * GW], BF16, kind="ExternalInput")
    Wo = nc.dram_tensor("Wo", [P, 2 * GW], BF16, kind="ExternalInput")
    W1 = nc.dram_tensor("W1", [P, 8 * GW], BF16, kind="ExternalInput")
    W2 = nc.dram_tensor("W2", [P, 8 * GW], BF16, kind="ExternalInput")
    Wada = nc.dram_tensor("Wada", [P, 12 * GW], BF16, kind="ExternalInput")
    bada_r = nc.dram_tensor("bada_r", [1, 6 * D], BF16, kind="ExternalInput")
    bq_c = nc.dram_tensor("bq_c", [P, KT], F32, kind="ExternalInput")  # prescaled
    bk_c = nc.dram_tensor("bk_c", [P, KT], F32, kind="ExternalInput")
    bo_c = nc.dram_tensor("bo_c", [P, KT], F32, kind="ExternalInput")  # bo_eff
    b1_c = nc.dram_tensor("b1_c", [P, MT], F32, kind="ExternalInput")
    b2_c = nc.dram_tensor("b2_c", [P, KT], F32, kind="ExternalInput")
    yT = nc.dram_tensor("yT", [D, NQ], F32, kind="ExternalOutput")


    with tile.TileContext(nc) as tc, ExitStack() as root:
        const = root.enter_context(tc.tile_pool(name="const", bufs=1))
        rootrows = root.enter_context(tc.tile_pool(name="rootrows", bufs=1))

        ones_col = const.tile([P, 1], BF16, name='ones_col')
        nc.gpsimd.memset(ones_col[:], 1.0)
        ones_row = const.tile([1, NQ], BF16, name='ones_row')
        nc.gpsimd.memset(ones_row[:], 1.0)
        eps_t = const.tile([1, 1], F32, name='eps_t')
        nc.gpsimd.memset(eps_t[:], EPS)

        bqT_s = const.tile([P, KT], F32, name='bqT_s')
        bkT = const.tile([P, KT], F32, name='bkT')
        boT = const.tile([P, KT], F32, name='boT')
        b1T = const.tile([P, MT], F32, name='b1T')
        b2T = const.tile([P, KT], F32, name='b2T')
        bada_sb = const.tile([1, 6 * D], BF16, name='bada_sb')

        csT = const.tile([P, KT], BF16, name='csT')
        gmsaT = const.tile([P, KT], F32, name='gmsaT')
        gmlpT = const.tile([P, KT], F32, name='gmlpT')
        mod_row = rootrows.tile([1, 6 * D], BF16, name='mod_row')

        # persistent activation arrays (left stack, LIFO close order)
        xp_cm = tc.tile_pool(name="xp", bufs=1, side='left')
        xp = xp_cm.__enter__()
        xall = xp.tile([P, KT * N], BF16, tag="xall", name="xall")
        xt = [xall[:, k * N:(k + 1) * N] for k in range(KT)]
        op_cm = tc.tile_pool(name="op", bufs=1, side='left')
        op_ = op_cm.__enter__()
        outT = [op_.tile([P, NQ], BF16, tag=f"o{k}", name=f"o{k}")
                for k in range(KT)]
        # Wada group tiles ([P, 4096]) - alive until mod group 11
        wada_cm = tc.tile_pool(name="wadap", bufs=3, side='left')
        wadap = wada_cm.__enter__()
        hT_cm = tc.tile_pool(name="hTp", bufs=1, side='left')
        hTp = hT_cm.__enter__()
        hT = [hTp.tile([P, N], BF16, tag=f"h{k}", name=f"h{k}")
              for k in range(KT)]

        nc.sync.dma_start(bada_sb[:], bada_r[:])
        for j in range(KT):
            eng = nc.scalar if j % 2 else nc.sync
            eng.dma_start(xall[:, j * N:(j + 1) * N],
                          xT[:, j * N:(j + 1) * N])
        nc.gpsimd.dma_start(bqT_s[:], bq_c[:])
        nc.gpsimd.dma_start(bkT[:], bk_c[:])
        nc.gpsimd.dma_start(boT[:], bo_c[:])
        nc.gpsimd.dma_start(b1T[:], b1_c[:])
        nc.gpsimd.dma_start(b2T[:], b2_c[:])

        wada_tiles = {}

        def load_wada_group(g, eng):
            wch = wadap.tile([P, KT * NQ], BF16, tag="wada", name='wada')
            eng.dma_start(wch[:], Wada[:, g * GW:(g + 1) * GW])
            wada_tiles[g] = wch

        def mod_group(g, psmod):
            """mod[:, g*512:(g+1)*512] = silu(c) @ Wada[:, gslice] + bada."""
            if g not in wada_tiles:
                load_wada_group(g, nc.gpsimd if g % 2 else nc.sync)
            wch = wada_tiles[g]
            mp = psmod.tile([1, NQ], F32, tag="modp", name='modp')
            for k in range(KT):
                nc.tensor.matmul(
                    mp[:], lhsT=csT[:, k:k + 1],
                    rhs=wch[:, k * NQ:(k + 1) * NQ],
                    start=(k == 0), stop=(k == KT - 1))
            nc.vector.tensor_add(mod_row[0:1, g * NQ:(g + 1) * NQ], mp[:],
                                 bada_sb[0:1, g * NQ:(g + 1) * NQ])

        def cols_from_row(row_ap, dst, ps, scale=None):
            """[1, n*128] row -> [128, n] column tile via K=1 matmuls."""
            n = dst.shape[-1]
            for j in range(n):
                nc.tensor.matmul(ps[:, j:j + 1],
                                 lhsT=row_ap[0:1, j * P:(j + 1) * P],
                                 rhs=ones_row[0:1, 0:1],
                                 start=True, stop=True)
            if scale is None:
                nc.vector.tensor_copy(dst[:], ps[:, 0:n])
            else:
                nc.vector.tensor_scalar_mul(dst[:], ps[:, 0:n], scale)

        # ---------------- phase 0+2: ln1, mod, modulate, q/k/v ----------
        qkv_cm = tc.tile_pool(name="qkvp", bufs=1, side='right')
        qkvp = qkv_cm.__enter__()
        qTt = [qkvp.tile([P, NQ], BF16, tag=f"q{k}", name=f"q{k}")
               for k in range(KT)]
        kTt = [qkvp.tile([P, N], BF16, tag=f"k{k}", name=f"k{k}")
               for k in range(KT)]
        vRt = [qkvp.tile([P, H * (DH + 1)], BF16, tag=f"v{k}", name=f"v{k}")
               for k in range(KT)]
        wkv_cm = tc.tile_pool(name="wkvp", bufs=1, side='right')
        wkvp = wkv_cm.__enter__()
        kg_t = wkvp.tile([P, 2 * KT * NQ], BF16, tag="kg", name="kg")
        vg_t = wkvp.tile([P, 2 * KT * NQ], BF16, tag="vg", name="vg")
        nc.sync.dma_start(kg_t[:, 0:GW], Wkv[:, 0:GW])
        nc.scalar.dma_start(kg_t[:, GW:2 * GW], Wkv[:, GW:2 * GW])
        nc.sync.dma_start(vg_t[:, 0:GW], Wkv[:, 2 * GW:3 * GW])
        nc.scalar.dma_start(vg_t[:, GW:2 * GW], Wkv[:, 3 * GW:4 * GW])

        def emit_kT(t, psp, wide=True):
            """k^T tile t over the full sequence."""
            g, dot = t // 4, t % 4
            base = g * GW
            if wide:
                pk = psp.tile([P, N], F32, tag="kprj", name='kprj')
                for ch in range(2):
                    sl = slice(ch * NQ, (ch + 1) * NQ)
                    for k in range(KT):
                        o = base + k * NQ + dot * P
                        nc.tensor.matmul(
                            pk[:, sl], lhsT=kg_t[:, o:o + P],
                            rhs=hT[k][:, sl],
                            start=(k == 0), stop=(k == KT - 1))
                nc.scalar.activation(kTt[t][:], pk[:], AF.Identity,
                                     bias=bkT[:, t:t + 1])
            else:
                for ch in range(2):
                    sl = slice(ch * NQ, (ch + 1) * NQ)
                    pk = psp.tile([P, NQ], F32, tag="vprj", name='kprjn')
                    for k in range(KT):
                        o = base + k * NQ + dot * P
                        nc.tensor.matmul(
                            pk[:], lhsT=kg_t[:, o:o + P],
                            rhs=hT[k][:, sl],
                            start=(k == 0), stop=(k == KT - 1))
                    nc.scalar.activation(kTt[t][:, sl], pk[:], AF.Identity,
                                         bias=bkT[:, t:t + 1])

        def emit_v(vg, nt, psp):
            base = vg * GW
            p = psp.tile([P, NQ], F32, tag="vprj", name='vprj')
            for k in range(KT):
                o = base + k * NQ
                nc.tensor.matmul(
                    p[:], lhsT=hT[k][:, nt * P:(nt + 1) * P],
                    rhs=vg_t[:, o:o + NQ],
                    start=(k == 0), stop=(k == KT - 1))
            vv = vRt[nt].rearrange("p (h w) -> p h w", w=DH + 1)
            pv = p.rearrange("p (h w) -> p h w", w=DH)
            nc.vector.tensor_copy(vv[:, vg * 8:(vg + 1) * 8, 0:DH], pv[:])

        with ExitStack() as ph:
            rows = ph.enter_context(tc.tile_pool(name="p0rows", bufs=1))
            psmod = ph.enter_context(
                tc.tile_pool(name="psmod", bufs=2, space="PSUM"))
            pscol = ph.enter_context(
                tc.tile_pool(name="pscol", bufs=1, space="PSUM"))

            # silu(c) and its column layout
            c_sb = rows.tile([1, D], BF16, name='c_sb')
            nc.scalar.dma_start(c_sb[:], crow[:])
            cs_row = rows.tile([1, D], BF16, name='cs_row')
            nc.scalar.activation(cs_row[:], c_sb[:], AF.Silu)
            pc = pscol.tile([P, KT], F32, tag="colps", name='colps')
            cols_from_row(cs_row, csT, pc)

            load_wada_group(0, nc.sync)
            load_wada_group(1, nc.scalar)
            load_wada_group(2, nc.sync)
            load_wada_group(3, nc.scalar)

            a_bf = rows.tile([1, N], BF16, name='a_bf')
            bstack = rows.tile([2, N], BF16, name='bstack')
            nc.gpsimd.memset(bstack[:, :], 1.0)  # row1 stays ones

            with ExitStack() as sec:
                psstat = sec.enter_context(
                    tc.tile_pool(name="psstat", bufs=2, space="PSUM"))
                sqpool = sec.enter_context(tc.tile_pool(name="p0sq",
                                                        bufs=2))
                for ch in range(2):
                    sl = slice(ch * NQ, (ch + 1) * NQ)
                    ss = psstat.tile([1, NQ], F32, tag="st_s", name='st_s')
                    sq_ps = psstat.tile([1, NQ], F32, tag="st_q",
                                        name='st_q')
                    for k in range(KT):
                        sq = sqpool.tile([P, NQ], BF16, tag="xsq",
                                         name='xsq')
                        nc.scalar.activation(
                            sq[:],
                            xall[:, k * N + ch * NQ:k * N + (ch + 1) * NQ],
                            AF.Square)
                        nc.tensor.matmul(
                            ss[:], lhsT=ones_col[:],
                            rhs=xall[:, k * N + ch * NQ:
                                     k * N + (ch + 1) * NQ],
                            start=(k == 0), stop=(k == KT - 1))
                        nc.tensor.matmul(sq_ps[:], lhsT=ones_col[:],
                                         rhs=sq[:],
                                         start=(k == 0), stop=(k == KT - 1))
                    t_row = rows.tile([1, NQ], F32, tag="t_row",
                                      name='t_row')
                    nc.scalar.activation(t_row[:], ss[:], AF.Square)
                    u_row = rows.tile([1, NQ], F32, tag="u_row",
                                      name='u_row')
                    nc.vector.scalar_tensor_tensor(
                        u_row[:], t_row[:], -1.0 / D, sq_ps[:],
                        ALU.mult, ALU.add)
                    sd_row = rows.tile([1, NQ], F32, tag="sd_row",
                                       name='sd_row')
                    nc.scalar.activation(sd_row[:], u_row[:], AF.Sqrt,
                                         bias=eps_t[:], scale=1.0 / D)
                    a32 = rows.tile([1, NQ], F32, tag="a32", name='a32')
                    nc.vector.reciprocal_approx_fast(a32[:], sd_row[:])
                    nc.vector.tensor_copy(a_bf[0:1, sl], a32[:])
                    b32 = rows.tile([1, NQ], F32, tag="b32", name='b32')
                    nc.vector.scalar_tensor_tensor(
                        b32[:], ss[:], -1.0 / D, a32[:], ALU.mult,
                        ALU.mult)
                    nc.vector.tensor_copy(bstack[0:1, sl], b32[:])

            # mod groups 0-3 (sh_msa, sc_msa) — PE filler while rows run
            for g in range(4):
                mod_group(g, psmod)

            # S1 = 1 + sc_msa (row 0), sh_msa (row 1, cross-partition DMA)
            S1sh1 = rows.tile([2, D], BF16, name='S1sh1')
            nc.vector.tensor_scalar_add(S1sh1[0:1, :], mod_row[0:1, D:2 * D],
                                        1.0)
            nc.scalar.dma_start(S1sh1[1:2, :], mod_row[0:1, 0:D])

            # h = x*A + B  (ch0 first so q can start early)
            with ExitStack() as sec:
                psab = sec.enter_context(
                    tc.tile_pool(name="psab", bufs=2, space="PSUM"))
                for ch in range(2):
                    for k in range(KT):
                        sl = slice(ch * NQ, (ch + 1) * NQ)
                        pa = psab.tile([P, NQ], F32, tag="pA", name='pA')
                        pb = psab.tile([P, NQ], F32, tag="pB", name='pB')
                        nc.tensor.matmul(
                            pa[:], lhsT=S1sh1[0:1, k * P:(k + 1) * P],
                            rhs=a_bf[0:1, sl], start=True, stop=True)
                        nc.tensor.matmul(
                            pb[:], lhsT=S1sh1[:, k * P:(k + 1) * P],
                            rhs=bstack[:, sl], start=True, stop=True)
                        nc.vector.tensor_mul(
                            hT[k][:, sl],
                            xall[:, k * N + ch * NQ:k * N + (ch + 1) * NQ],
                            pa[:])
                        nc.vector.tensor_add(hT[k][:, sl], hT[k][:, sl],
                                             pb[:])

        # phase 2: q, k0/1, v(vg0)
        with ExitStack() as ph:
            wqpool = ph.enter_context(tc.tile_pool(name="p2wq", bufs=1))
            ps = ph.enter_context(
                tc.tile_pool(name="p2ps", bufs=2, space="PSUM"))
            psk = ph.enter_context(
                tc.tile_pool(name="p2psk", bufs=1, space="PSUM"))

            wq_t = wqpool.tile([P, 2 * KT * NQ], BF16, tag="wq", name="wq")
            nc.sync.dma_start(wq_t[:, 0:GW], Wq[:, 0:GW])
            nc.scalar.dma_start(wq_t[:, GW:2 * GW], Wq[:, GW:2 * GW])

            for nt in range(KT):
                vv = vRt[nt].rearrange("p (h w) -> p h w", w=DH + 1)
                nc.gpsimd.memset(vv[:, :, DH:DH + 1], 1.0)

            for g in range(2):
                for dot in range(4):
                    t = 4 * g + dot
                    p = ps.tile([P, NQ], F32, tag="qprj", name='qprj')
                    for k in range(KT):
                        o = g * GW + k * NQ + dot * P
                        nc.tensor.matmul(
                            p[:], lhsT=wq_t[:, o:o + P],
                            rhs=hT[k][:, 0:NQ],
                            start=(k == 0), stop=(k == KT - 1))
                    nc.scalar.activation(qTt[t][:], p[:], AF.Identity,
                                         bias=bqT_s[:, t:t + 1],
                                         scale=DH ** -0.5)

            emit_kT(0, psk)
            emit_kT(1, psk)
            for nt in range(6):
                emit_v(0, nt, ps)

        # ---------------- phase 3: attention ----------------
        with ExitStack() as ph:
            epool = ph.enter_context(tc.tile_pool(name="p3e", bufs=10))
            spool = ph.enter_context(tc.tile_pool(name="p3s", bufs=1))
            ps_sim = ph.enter_context(
                tc.tile_pool(name="ps_sim", bufs=2, space="PSUM"))
            ps_o = ph.enter_context(
                tc.tile_pool(name="ps_o", bufs=1, space="PSUM"))
            ps_bc = ph.enter_context(
                tc.tile_pool(name="ps_bc", bufs=1, space="PSUM"))
            ps_jit = ph.enter_context(
                tc.tile_pool(name="ps_jit", bufs=1, space="PSUM"))
            ps_modp = ph.enter_context(
                tc.tile_pool(name="ps_modp", bufs=1, space="PSUM"))


            def emit_kT_ch(t, ch, psp):
                g, dot = t // 4, t % 4
                base = g * GW
                sl = slice(ch * NQ, (ch + 1) * NQ)
                pk = psp.tile([P, NQ], F32, tag="vprj", name='kprjn')
                for k in range(KT):
                    o = base + k * NQ + dot * P
                    nc.tensor.matmul(
                        pk[:], lhsT=kg_t[:, o:o + P],
                        rhs=hT[k][:, sl],
                        start=(k == 0), stop=(k == KT - 1))
                nc.scalar.activation(kTt[t][:, sl], pk[:], AF.Identity,
                                     bias=bkT[:, t:t + 1])

            for hp in range(H // 2):
                pt = hp
                # JIT work units to weave between sim/exp pairs
                jit = []
                if 0 < hp < 7:
                    jit.append(lambda t=hp + 1: emit_kT_ch(t, 0, ps_jit))
                    jit.append(lambda t=hp + 1: emit_kT_ch(t, 1, ps_jit))
                if hp == 0:
                    jit.append(lambda: emit_v(0, 6, ps_jit))
                    jit.append(lambda: emit_v(0, 7, ps_jit))
                if hp < 4:
                    jit.append(lambda nt=2 * hp: emit_v(1, nt, ps_jit))
                    jit.append(lambda nt=2 * hp + 1: emit_v(1, nt, ps_jit))
                if 1 <= hp <= 6:
                    jit.append(lambda g=3 + hp: mod_group(g, ps_modp))
                if hp == 6:
                    jit.append(lambda: mod_group(10, ps_modp))
                if hp == 7:
                    jit.append(lambda: mod_group(11, ps_modp))
                if hp == 2:
                    def _gmsa():
                        pg = ps_jit.tile([P, NQ], F32, tag="vprj",
                                         name='gcol')
                        cols_from_row(mod_row[0:1, 2 * D:3 * D], gmsaT, pg)
                    jit.append(_gmsa)
                if hp == 7:
                    def _gmlp():
                        pg = ps_jit.tile([P, NQ], F32, tag="vprj",
                                         name='gcol2')
                        cols_from_row(mod_row[0:1, 5 * D:6 * D], gmlpT, pg)
                    jit.append(_gmlp)

                et = {}
                ji = 0
                start_slot = max(0, 8 - len(jit))
                for hi in range(2):
                    hh = hi * DH
                    for kp in range(4):
                        psim = ps_sim.tile([P, N], F32, tag="sim",
                                           name='sim')
                        for j in range(2):
                            kt = 2 * kp + j
                            nc.tensor.matmul(
                                psim[:, j * NQ:(j + 1) * NQ],
                                lhsT=kTt[pt][hh:hh + DH,
                                             kt * P:(kt + 1) * P],
                                rhs=qTt[pt][hh:hh + DH, :],
                                start=True, stop=True)
                        e = epool.tile([P, N], BF16, tag="e", name='e')
                        nc.scalar.activation(e[:], psim[:], AF.Exp)
                        et[hi, kp] = e
                        if hi * 4 + kp >= start_slot and ji < len(jit):
                            jit[ji]()
                            ji += 1
                while ji < len(jit):
                    jit[ji]()
                    ji += 1
                for hi in range(2):
                    h = 2 * hp + hi
                    hh = hi * DH
                    pos = ps_o.tile([DH + 1, NQ], F32, tag="ov", name='ov')
                    for kp in range(4):
                        for j in range(2):
                            kt = 2 * kp + j
                            nc.tensor.matmul(
                                pos[:],
                                lhsT=vRt[kt][:, h * (DH + 1):
                                             (h + 1) * (DH + 1)],
                                rhs=et[hi, kp][:, j * NQ:(j + 1) * NQ],
                                start=(kt == 0), stop=(kt == KT - 1))
                    # normalize: inv = 1/denom (fast recip), PE broadcast
                    den = spool.tile([1, NQ], F32, tag="den", name='den')
                    nc.vector.tensor_copy(den[:], pos[DH:DH + 1, :])
                    inv32 = spool.tile([1, NQ], F32, tag="inv32",
                                       name='inv32')
                    nc.vector.reciprocal_approx_fast(inv32[:], den[:])
                    inv16 = spool.tile([1, NQ], BF16, tag="inv16",
                                       name='inv16')
                    nc.vector.tensor_copy(inv16[:], inv32[:])
                    pb = ps_bc.tile([DH, NQ], F32, tag="bc", name='bc')
                    nc.tensor.matmul(pb[:], lhsT=ones_row[0:1, 0:DH],
                                     rhs=inv16[:], start=True, stop=True)
                    binv = spool.tile([DH, NQ], F32, tag="binv",
                                      name='binv')
                    nc.vector.tensor_copy(binv[:], pb[:])
                    nc.vector.tensor_mul(outT[pt][hh:hh + DH, :],
                                         pos[0:DH, :], binv[:])

        wkv_cm.__exit__(None, None, None)
        qkv_cm.__exit__(None, None, None)
        hT_cm.__exit__(None, None, None)
        wada_cm.__exit__(None, None, None)

        # ---------------- phase 4: Wo + residual + ln2 ----------------
        wpool = root.enter_context(tc.tile_pool(name="p5w", bufs=2,
                                                side='right'))
        wopool_r = root.enter_context(tc.tile_pool(name="p4wo", bufs=1,
                                                   side='right'))
        wo_t = wopool_r.tile([P, 2 * KT * NQ], BF16, tag="wo", name='wo')
        nc.sync.dma_start(wo_t[:, 0:GW], Wo[:, 0:GW])
        nc.sync.dma_start(wo_t[:, GW:2 * GW], Wo[:, GW:2 * GW])
        w1pair_tiles = {}
        w1_p0 = wpool.tile([P, 2 * KT * NQ], BF16, tag="w1", name='w1')
        nc.gpsimd.dma_start(w1_p0[:, 0:GW], W1[:, 0:GW])
        nc.sync.dma_start(w1_p0[:, GW:2 * GW], W1[:, GW:2 * GW])
        w1pair_tiles[0] = w1_p0
        x1p = root.enter_context(tc.tile_pool(name="x1p", bufs=1,
                                              side='right'))
        x1t = [x1p.tile([P, NQ], BF16, tag=f"x1{k}", name=f"x1{k}")
               for k in range(KT)]
        h2p = root.enter_context(tc.tile_pool(name="h2p", bufs=1,
                                              side='right'))
        h2t = [h2p.tile([P, NQ], BF16, tag=f"h2{k}", name=f"h2{k}")
               for k in range(KT)]
        x1bp = root.enter_context(tc.tile_pool(name="x1bp", bufs=1,
                                               side='right'))
        x1b = [x1bp.tile([P, NQ], BF16, tag=f"x1b{k}", name=f"x1b{k}")
               for k in range(KT)]

        with ExitStack() as ph:
            rows4 = ph.enter_context(tc.tile_pool(name="p4rows", bufs=1))
            psy = ph.enter_context(
                tc.tile_pool(name="psy", bufs=2, space="PSUM"))

            # xo2[t] = x_own + gmsa*bo_eff  (per-partition col broadcast)
            bo2 = rows4.tile([P, KT], F32, name='bo2')
            nc.vector.tensor_mul(bo2[:], boT[:], gmsaT[:])
            xo2 = [rows4.tile([P, NQ], BF16, tag=f"xo2{k}", name=f"xo2{k}")
                   for k in range(KT)]
            for k in range(KT):
                nc.scalar.activation(xo2[k][:], xall[:, k * N:k * N + NQ],
                                     AF.Identity, bias=bo2[:, k:k + 1])

            psstat2 = ph.enter_context(
                tc.tile_pool(name="psstat2", bufs=1, space="PSUM"))
            sq2pool = ph.enter_context(tc.tile_pool(name="p4sq", bufs=4))
            ss2 = psstat2.tile([1, NQ], F32, tag="st2s", name='st2s')
            sq2_ps = psstat2.tile([1, NQ], F32, tag="st2q", name='st2q')
            for g in range(2):
                for dot in range(4):
                    t = 4 * g + dot
                    p = psy.tile([P, NQ], F32, tag="y1", name='y1')
                    for k in range(KT):
                        o = g * GW + k * NQ + dot * P
                        nc.tensor.matmul(
                            p[:], lhsT=wo_t[:, o:o + P],
                            rhs=outT[k][:],
                            start=(k == 0), stop=(k == KT - 1))
                    nc.vector.scalar_tensor_tensor(
                        x1t[t][:], p[:], gmsaT[:, t:t + 1], xo2[t][:],
                        ALU.mult, ALU.add)
                    sq = sq2pool.tile([P, NQ], BF16, tag="x1sq",
                                      name='x1sq')
                    nc.scalar.activation(sq[:], x1t[t][:], AF.Square)
                    nc.tensor.matmul(ss2[:], lhsT=ones_col[:],
                                     rhs=x1t[t][:],
                                     start=(t == 0), stop=(t == KT - 1))
                    nc.tensor.matmul(sq2_ps[:], lhsT=ones_col[:],
                                     rhs=sq[:],
                                     start=(t == 0), stop=(t == KT - 1))

            # ln2 rows (stats accumulated inside the Wo loop above)
            a2_bf = rows4.tile([1, NQ], BF16, name='a2_bf')
            b2stack = rows4.tile([2, NQ], BF16, name='b2stack')
            nc.gpsimd.memset(b2stack[:, :], 1.0)  # row1 stays ones
            t_row = rows4.tile([1, NQ], F32, name='t2_row')
            nc.scalar.activation(t_row[:], ss2[:], AF.Square)
            u_row = rows4.tile([1, NQ], F32, name='u2_row')
            nc.vector.scalar_tensor_tensor(
                u_row[:], t_row[:], -1.0 / D, sq2_ps[:], ALU.mult,
                ALU.add)
            sd_row = rows4.tile([1, NQ], F32, name='sd2_row')
            nc.scalar.activation(sd_row[:], u_row[:], AF.Sqrt,
                                 bias=eps_t[:], scale=1.0 / D)
            a32 = rows4.tile([1, NQ], F32, name='a232')
            nc.vector.reciprocal_approx_fast(a32[:], sd_row[:])
            nc.vector.tensor_copy(a2_bf[:], a32[:])
            b32 = rows4.tile([1, NQ], F32, name='b232')
            nc.vector.scalar_tensor_tensor(
                b32[:], ss2[:], -1.0 / D, a32[:], ALU.mult, ALU.mult)
            nc.vector.tensor_copy(b2stack[0:1, :], b32[:])

            S2sh2 = rows4.tile([2, D], BF16, name='S2sh2')
            nc.vector.tensor_scalar_add(S2sh2[0:1, :],
                                        mod_row[0:1, 4 * D:5 * D], 1.0)
            nc.scalar.dma_start(S2sh2[1:2, :], mod_row[0:1, 3 * D:4 * D])

            # x1b[t] = x1 + gmlp*b2  (for the MLP epilogue)
            b2g = rows4.tile([P, KT], F32, name='b2g')
            nc.vector.tensor_mul(b2g[:], b2T[:], gmlpT[:])
            for k in range(KT):
                nc.scalar.activation(x1b[k][:], x1t[k][:], AF.Identity,
                                     bias=b2g[:, k:k + 1])

            with ExitStack() as sec:
                psab2 = sec.enter_context(
                    tc.tile_pool(name="psab2", bufs=2, space="PSUM"))
                for k in range(KT):
                    pa = psab2.tile([P, NQ], F32, tag="pA2", name='pA2')
                    pb = psab2.tile([P, NQ], F32, tag="pB2", name='pB2')
                    nc.tensor.matmul(
                        pa[:], lhsT=S2sh2[0:1, k * P:(k + 1) * P],
                        rhs=a2_bf[:], start=True, stop=True)
                    nc.tensor.matmul(
                        pb[:], lhsT=S2sh2[:, k * P:(k + 1) * P],
                        rhs=b2stack[:], start=True, stop=True)
                    nc.vector.tensor_mul(h2t[k][:], x1t[k][:], pa[:])
                    nc.vector.tensor_add(h2t[k][:], h2t[k][:], pb[:])

        op_cm.__exit__(None, None, None)
        xp_cm.__exit__(None, None, None)

        # ---------------- phase 5: MLP ----------------
        with ExitStack() as ph:
            gp = ph.enter_context(tc.tile_pool(name="gp", bufs=1))
            gTt = [gp.tile([P, NQ], BF16, tag=f"g{m}", name=f"g{m}")
                   for m in range(MT)]
            w2pool = ph.enter_context(tc.tile_pool(name="p5w2", bufs=2))
            opool = ph.enter_context(tc.tile_pool(name="p5o", bufs=3))
            ps1 = ph.enter_context(
                tc.tile_pool(name="ps1", bufs=3, space="PSUM"))
            ps2 = ph.enter_context(
                tc.tile_pool(name="ps2", bufs=1, space="PSUM"))

            for gp2 in range(4):   # 4 pair loads of 2 groups
                if gp2 in w1pair_tiles:
                    w1_g = w1pair_tiles[gp2]
                else:
                    w1_g = wpool.tile([P, 2 * KT * NQ], BF16, tag="w1",
                                      name='w1')
                    eng = nc.gpsimd if gp2 % 2 else nc.sync
                    eng2 = nc.sync if gp2 % 2 else nc.gpsimd
                    eng.dma_start(w1_g[:, 0:GW],
                                  W1[:, 2 * gp2 * GW:(2 * gp2 + 1) * GW])
                    eng2.dma_start(w1_g[:, GW:2 * GW],
                                   W1[:, (2 * gp2 + 1) * GW:
                                      (2 * gp2 + 2) * GW])
                for gh in range(2):
                    g = 2 * gp2 + gh
                    for dot in range(4):
                        m = 4 * g + dot
                        p = ps1.tile([P, NQ], F32, tag="m1", name='m1')
                        for k in range(KT):
                            o = gh * GW + k * NQ + dot * P
                            nc.tensor.matmul(
                                p[:], lhsT=w1_g[:, o:o + P],
                                rhs=h2t[k][:],
                                start=(k == 0), stop=(k == KT - 1))
                        nc.scalar.activation(gTt[m][:], p[:],
                                             AF.Gelu_apprx_tanh,
                                             bias=b1T[:, m:m + 1])

            for half in range(2):
                pacc = [ps2.tile([P, NQ], F32, tag=f"acc{d}",
                                 name=f"acc{d}") for d in range(4)]
                for mg2 in range(2):   # 2 pair loads of 16 k-chunks
                    w2c = w2pool.tile([P, 2 * KT * NQ], BF16, tag="w2",
                                      name='w2')
                    blk = half * 4 + 2 * mg2
                    eng = nc.gpsimd if (half + mg2) % 2 else nc.sync
                    eng2 = nc.sync if (half + mg2) % 2 else nc.gpsimd
                    eng.dma_start(w2c[:, 0:GW],
                                  W2[:, blk * GW:(blk + 1) * GW])
                    eng2.dma_start(w2c[:, GW:2 * GW],
                                   W2[:, (blk + 1) * GW:(blk + 2) * GW])
                    for kk in range(2 * KT):
                        mk = 2 * mg2 * KT + kk
                        for d in range(4):
                            o = kk * NQ + d * P
                            nc.tensor.matmul(
                                pacc[d][:], lhsT=w2c[:, o:o + P],
                                rhs=gTt[mk][:],
                                start=(mk == 0), stop=(mk == MT - 1))
                for d in range(4):
                    t = half * 4 + d
                    yt = opool.tile([P, NQ], F32, tag="yout", name='yout')
                    nc.vector.scalar_tensor_tensor(
                        yt[:], pacc[d][:], gmlpT[:, t:t + 1], x1b[t][:],
                        ALU.mult, ALU.add)
                    nc.sync.dma_start(yT[t * P:(t + 1) * P, :], yt[:])

    nc.compile()
    return nc


_NC = None


def _get_nc():
    global _NC
    if _NC is None:
        _NC = build()
    return _NC


def _pack_cols(W, bf):
    """[Din, C] -> [P, (C//512)*Din*... ] device-tile-contiguous layout."""
    Din, C = W.shape
    kt = Din // P
    blocks = [W.reshape(kt, P, C)[:, :, g * 512:(g + 1) * 512]
              .transpose(1, 0, 2).reshape(P, kt * 512)
              for g in range(C // 512)]
    return np.ascontiguousarray(np.concatenate(blocks, axis=1)).astype(bf)


def _prep_inputs(x, c, Wq, bq, Wkv, bkv, Wo, bo, W1, b1, W2, b2, Wada, bada):
    import ml_dtypes
    f = np.float32
    bf = ml_dtypes.bfloat16
    col = lambda v, n: np.ascontiguousarray(
        np.asarray(v, f).reshape(n, P).T)
    bkv = np.asarray(bkv, f)
    Wo_f = np.asarray(Wo, f)
    bo_eff = np.asarray(bo, f) + bkv[D:] @ Wo_f   # fold v-bias through Wo
    W2f = np.asarray(W2, f)
    w2blocks = [W2f.reshape(MT, P, D)[mg * 8:(mg + 1) * 8, :,
                                      half * 512:(half + 1) * 512]
                .transpose(1, 0, 2).reshape(P, KT * 512)
                for half in range(2) for mg in range(4)]
    shared = {
        "Wq": _pack_cols(np.asarray(Wq, f), bf),
        "Wkv": _pack_cols(np.asarray(Wkv, f), bf),
        "Wo": _pack_cols(Wo_f, bf),
        "W1": _pack_cols(np.asarray(W1, f), bf),
        "W2": np.ascontiguousarray(
            np.concatenate(w2blocks, axis=1)).astype(bf),
        "Wada": _pack_cols(np.asarray(Wada, f), bf),
        "bada_r": np.asarray(bada, f).reshape(1, -1).astype(bf),
        "bq_c": col(np.asarray(bq, f) * (DH ** -0.5), KT),
        "bk_c": col(bkv[:D], KT),
        "bo_c": col(bo_eff, KT),
        "b1_c": col(b1, MT), "b2_c": col(b2, KT),
    }
    in_maps = []
    for core in range(NCORES):
        b, half = core // 2, core % 2
        xb = np.asarray(x[b], f)
        perm = np.concatenate(
            [xb[half * NQ:(half + 1) * NQ],
             xb[(1 - half) * NQ:(2 - half) * NQ]], axis=0)
        m = dict(shared)
        px = perm.T  # [D, N] feature-major
        m["xT"] = np.ascontiguousarray(
            np.concatenate([px[k * P:(k + 1) * P, :] for k in range(KT)],
                           axis=1)).astype(bf)
        m["crow"] = np.asarray(c[b:b + 1], f).astype(bf)
        in_maps.append(m)
    return in_maps


def _run(inputs, trace=False):
    nc = _get_nc()
    in_maps = _prep_inputs(**inputs)
    res = run_bass_kernel_spmd(nc, in_maps, core_ids=list(range(NCORES)),
                               trace=trace)
    B = 4
    y = np.empty((B, N, D), np.float32)
    for core in range(NCORES):
        b, half = core // 2, core % 2
        y[b, half * NQ:(half + 1) * NQ, :] = res.results[core]["yT"].T
    return y, res


def kernel(**inputs):
    y, _ = _run(inputs, trace=False)
    return y


# revision 45
# speedup vs baseline: 1.0258x; 1.0258x over previous
"""AdaLN attention block (DiT-style) on 8 TRN2 NeuronCores.

Sharding: 8 cores = 4 batches x 2 query-token halves, no collectives. Core c
handles batch c//2 and query half c%2: layernorm1 and k/v cover the full
(permuted) sequence; everything else covers only the own 512 query rows.

Device layout is feature-major (activations transposed, [d, n]). X @ W runs
with W column-tiles stationary and X^T moving, producing Y^T directly.
LayerNorm statistics use ones-vector matmuls (partition-axis sums on the PE);
the AdaLN modulate is h = x*A + B with rank-1 A/B built by K=1/K=2
outer-product matmuls into PSUM. Softmax skips max-subtraction (exp in fp32
psum, bf16 e tiles); the denominator is a ones-column appended to the attn@v
stationary operand; per-head normalization uses a fast DVE reciprocal and a
PE row-broadcast.

v2 changes vs the original baseline:
- x shipped bf16 (halves DMA, kills cast ops); bo folded with the v-bias on
  host (bo_eff = bo + bv @ Wo).
- exp runs on 1024-wide psum tensors (half the Act-engine dispatch cost).
- softmax/LN reciprocals use reciprocal_approx_fast/accurate (5x faster).
- elementwise work split across DVE and Pool (gpsimd) engines.
- epilogues fused into single scalar_tensor_tensor ops.
- Wada/mod matmul groups 4-11 are interleaved into the attention loop where
  the PE otherwise waits on exp results.
- weights arrive via merged ~1MB DMAs (one per 512-column group).
"""

import numpy as np
from contextlib import ExitStack

import concourse.bass as bass
import concourse.bacc as bacc
import concourse.mybir as mybir
from concourse import tile
from concourse.bass_utils import run_bass_kernel_spmd

P = 128
D = 1024
N = 1024
NQ = 512
H = 16
DH = 64
MLPD = 4096
EPS = 1e-6
NCORES = 8

F32 = mybir.dt.float32
BF16 = mybir.dt.bfloat16
AF = mybir.ActivationFunctionType
ALU = mybir.AluOpType

KT = D // P           # 8 contraction tiles over D
MT = MLPD // P        # 32 tiles over MLP dim


def build():
    nc = bacc.Bacc("TRN2", target_bir_lowering=False, debug=False,
                   num_devices=NCORES)

    GW = KT * NQ   # 4096 packed columns per 512-wide output group
    xT = nc.dram_tensor("xT", [P, KT * N], BF16, kind="ExternalInput")
    crow = nc.dram_tensor("crow", [1, D], BF16, kind="ExternalInput")
    Wq = nc.dram_tensor("Wq", [P, 2 * GW], BF16, kind="ExternalInput")
    Wkv = nc.dram_tensor("Wkv", [P, 4 * GW], BF16, kind="ExternalInput")
    Wo = nc.dram_tensor("Wo", [P, 2 * GW], BF16, kind="ExternalInput")
    W1 = nc.dram_tensor("W1", [P, 8 * GW], BF16, kind="ExternalInput")
    W2 = nc.dram_tensor("W2", [P, 8 * GW], BF16, kind="ExternalInput")
    Wada = nc.dram_tensor("Wada", [P, 12 * GW], BF16, kind="ExternalInput")
    bada_r = nc.dram_tensor("bada_r", [1, 6 * D], BF16, kind="ExternalInput")
    bq_c = nc.dram_tensor("bq_c", [P, KT], F32, kind="ExternalInput")  # prescaled
    bk_c = nc.dram_tensor("bk_c", [P, KT], F32, kind="ExternalInput")
    bo_c = nc.dram_tensor("bo_c", [P, KT], F32, kind="ExternalInput")  # bo_eff
    b1_c = nc.dram_tensor("b1_c", [P, MT], F32, kind="ExternalInput")
    b2_c = nc.dram_tensor("b2_c", [P, KT], F32, kind="ExternalInput")
    yT = nc.dram_tensor("yT", [D, NQ], F32, kind="ExternalOutput")


    with tile.TileContext(nc) as tc, ExitStack() as root:
        const = root.enter_context(tc.tile_pool(name="const", bufs=1))
        rootrows = root.enter_context(tc.tile_pool(name="rootrows", bufs=1))

        ones_col = const.tile([P, 1], BF16, name='ones_col')
        nc.gpsimd.memset(ones_col[:], 1.0)
        ones_row = const.tile([1, NQ], BF16, name='ones_row')
        nc.gpsimd.memset(ones_row[:], 1.0)
        eps_t = const.tile([1, 1], F32, name='eps_t')
        nc.gpsimd.memset(eps_t[:], EPS)

        bqT_s = const.tile([P, KT], F32, name='bqT_s')
        bkT = const.tile([P, KT], F32, name='bkT')
        boT = const.tile([P, KT], F32, name='boT')
        b1T = const.tile([P, MT], F32, name='b1T')
        b2T = const.tile([P, KT], F32, name='b2T')
        bada_sb = const.tile([1, 6 * D], BF16, name='bada_sb')

        csT = const.tile([P, KT], BF16, name='csT')
        gmsaT = const.tile([P, KT], F32, name='gmsaT')
        gmlpT = const.tile([P, KT], F32, name='gmlpT')
        mod_row = rootrows.tile([1, 6 * D], BF16, name='mod_row')

        # persistent activation arrays (left stack, LIFO close order)
        xp_cm = tc.tile_pool(name="xp", bufs=1, side='left')
        xp = xp_cm.__enter__()
        xall = xp.tile([P, KT * N], BF16, tag="xall", name="xall")
        xt = [xall[:, k * N:(k + 1) * N] for k in range(KT)]
        op_cm = tc.tile_pool(name="op", bufs=1, side='left')
        op_ = op_cm.__enter__()
        outT = [op_.tile([P, NQ], BF16, tag=f"o{k}", name=f"o{k}")
                for k in range(KT)]
        # Wada group tiles ([P, 4096]) - alive until mod group 11
        wada_cm = tc.tile_pool(name="wadap", bufs=3, side='left')
        wadap = wada_cm.__enter__()
        hT_cm = tc.tile_pool(name="hTp", bufs=1, side='left')
        hTp = hT_cm.__enter__()
        hT = [hTp.tile([P, N], BF16, tag=f"h{k}", name=f"h{k}")
              for k in range(KT)]

        nc.sync.dma_start(bada_sb[:], bada_r[:])
        for j in range(KT):
            eng = nc.scalar if j % 2 else nc.sync
            eng.dma_start(xall[:, j * N:(j + 1) * N],
                          xT[:, j * N:(j + 1) * N])
        nc.gpsimd.dma_start(bqT_s[:], bq_c[:])
        nc.gpsimd.dma_start(bkT[:], bk_c[:])
        nc.gpsimd.dma_start(boT[:], bo_c[:])
        nc.gpsimd.dma_start(b1T[:], b1_c[:])
        nc.gpsimd.dma_start(b2T[:], b2_c[:])

        wada_tiles = {}

        def load_wada_group(g, eng):
            wch = wadap.tile([P, KT * NQ], BF16, tag="wada", name='wada')
            eng.dma_start(wch[:], Wada[:, g * GW:(g + 1) * GW])
            wada_tiles[g] = wch

        def mod_group(g, psmod):
            """mod[:, g*512:(g+1)*512] = silu(c) @ Wada[:, gslice] + bada."""
            if g not in wada_tiles:
                load_wada_group(g, nc.gpsimd if g % 2 else nc.sync)
            wch = wada_tiles[g]
            mp = psmod.tile([1, NQ], F32, tag="modp", name='modp')
            for k in range(KT):
                nc.tensor.matmul(
                    mp[:], lhsT=csT[:, k:k + 1],
                    rhs=wch[:, k * NQ:(k + 1) * NQ],
                    start=(k == 0), stop=(k == KT - 1))
            nc.vector.tensor_add(mod_row[0:1, g * NQ:(g + 1) * NQ], mp[:],
                                 bada_sb[0:1, g * NQ:(g + 1) * NQ])

        def cols_from_row(row_ap, dst, ps, scale=None):
            """[1, n*128] row -> [128, n] column tile via K=1 matmuls."""
            n = dst.shape[-1]
            for j in range(n):
                nc.tensor.matmul(ps[:, j:j + 1],
                                 lhsT=row_ap[0:1, j * P:(j + 1) * P],
                                 rhs=ones_row[0:1, 0:1],
                                 start=True, stop=True)
            if scale is None:
                nc.vector.tensor_copy(dst[:], ps[:, 0:n])
            else:
                nc.vector.tensor_scalar_mul(dst[:], ps[:, 0:n], scale)

        # ---------------- phase 0+2: ln1, mod, modulate, q/k/v ----------
        qkv_cm = tc.tile_pool(name="qkvp", bufs=1, side='right')
        qkvp = qkv_cm.__enter__()
        qTt = [qkvp.tile([P, NQ], BF16, tag=f"q{k}", name=f"q{k}")
               for k in range(KT)]
        kTt = [qkvp.tile([P, N], BF16, tag=f"k{k}", name=f"k{k}")
               for k in range(KT)]
        vRt = [qkvp.tile([P, H * (DH + 1)], BF16, tag=f"v{k}", name=f"v{k}")
               for k in range(KT)]
        wkv_cm = tc.tile_pool(name="wkvp", bufs=1, side='right')
        wkvp = wkv_cm.__enter__()
        kg_t = wkvp.tile([P, 2 * KT * NQ], BF16, tag="kg", name="kg")
        vg_t = wkvp.tile([P, 2 * KT * NQ], BF16, tag="vg", name="vg")
        nc.sync.dma_start(kg_t[:, 0:GW], Wkv[:, 0:GW])
        nc.scalar.dma_start(kg_t[:, GW:2 * GW], Wkv[:, GW:2 * GW])
        nc.sync.dma_start(vg_t[:, 0:GW], Wkv[:, 2 * GW:3 * GW])
        nc.scalar.dma_start(vg_t[:, GW:2 * GW], Wkv[:, 3 * GW:4 * GW])

        def emit_kT(t, psp, wide=True):
            """k^T tile t over the full sequence."""
            g, dot = t // 4, t % 4
            base = g * GW
            if wide:
                pk = psp.tile([P, N], F32, tag="kprj", name='kprj')
                for ch in range(2):
                    sl = slice(ch * NQ, (ch + 1) * NQ)
                    for k in range(KT):
                        o = base + k * NQ + dot * P
                        nc.tensor.matmul(
                            pk[:, sl], lhsT=kg_t[:, o:o + P],
                            rhs=hT[k][:, sl],
                            start=(k == 0), stop=(k == KT - 1))
                nc.scalar.activation(kTt[t][:], pk[:], AF.Identity,
                                     bias=bkT[:, t:t + 1])
            else:
                for ch in range(2):
                    sl = slice(ch * NQ, (ch + 1) * NQ)
                    pk = psp.tile([P, NQ], F32, tag="vprj", name='kprjn')
                    for k in range(KT):
                        o = base + k * NQ + dot * P
                        nc.tensor.matmul(
                            pk[:], lhsT=kg_t[:, o:o + P],
                            rhs=hT[k][:, sl],
                            start=(k == 0), stop=(k == KT - 1))
                    nc.scalar.activation(kTt[t][:, sl], pk[:], AF.Identity,
                                         bias=bkT[:, t:t + 1])

        def emit_v(vg, nt, psp):
            base = vg * GW
            p = psp.tile([P, NQ], F32, tag="vprj", name='vprj')
            for k in range(KT):
                o = base + k * NQ
                nc.tensor.matmul(
                    p[:], lhsT=hT[k][:, nt * P:(nt + 1) * P],
                    rhs=vg_t[:, o:o + NQ],
                    start=(k == 0), stop=(k == KT - 1))
            vv = vRt[nt].rearrange("p (h w) -> p h w", w=DH + 1)
            pv = p.rearrange("p (h w) -> p h w", w=DH)
            nc.vector.tensor_copy(vv[:, vg * 8:(vg + 1) * 8, 0:DH], pv[:])

        with ExitStack() as ph:
            rows = ph.enter_context(tc.tile_pool(name="p0rows", bufs=1))
            psmod = ph.enter_context(
                tc.tile_pool(name="psmod", bufs=2, space="PSUM"))
            pscol = ph.enter_context(
                tc.tile_pool(name="pscol", bufs=1, space="PSUM"))

            # silu(c) and its column layout
            c_sb = rows.tile([1, D], BF16, name='c_sb')
            nc.scalar.dma_start(c_sb[:], crow[:])
            cs_row = rows.tile([1, D], BF16, name='cs_row')
            nc.scalar.activation(cs_row[:], c_sb[:], AF.Silu)
            pc = pscol.tile([P, KT], F32, tag="colps", name='colps')
            cols_from_row(cs_row, csT, pc)

            load_wada_group(0, nc.sync)
            load_wada_group(1, nc.scalar)
            load_wada_group(2, nc.sync)
            load_wada_group(3, nc.scalar)

            a_bf = rows.tile([1, N], BF16, name='a_bf')
            bstack = rows.tile([2, N], BF16, name='bstack')
            nc.gpsimd.memset(bstack[:, :], 1.0)  # row1 stays ones

            with ExitStack() as sec:
                psstat = sec.enter_context(
                    tc.tile_pool(name="psstat", bufs=2, space="PSUM"))
                sqpool = sec.enter_context(tc.tile_pool(name="p0sq",
                                                        bufs=2))
                for ch in range(2):
                    sl = slice(ch * NQ, (ch + 1) * NQ)
                    ss = psstat.tile([1, NQ], F32, tag="st_s", name='st_s')
                    sq_ps = psstat.tile([1, NQ], F32, tag="st_q",
                                        name='st_q')
                    for k in range(KT):
                        sq = sqpool.tile([P, NQ], BF16, tag="xsq",
                                         name='xsq')
                        nc.scalar.activation(
                            sq[:],
                            xall[:, k * N + ch * NQ:k * N + (ch + 1) * NQ],
                            AF.Square)
                        nc.tensor.matmul(
                            ss[:], lhsT=ones_col[:],
                            rhs=xall[:, k * N + ch * NQ:
                                     k * N + (ch + 1) * NQ],
                            start=(k == 0), stop=(k == KT - 1))
                        nc.tensor.matmul(sq_ps[:], lhsT=ones_col[:],
                                         rhs=sq[:],
                                         start=(k == 0), stop=(k == KT - 1))
                    t_row = rows.tile([1, NQ], F32, tag="t_row",
                                      name='t_row')
                    nc.scalar.activation(t_row[:], ss[:], AF.Square)
                    u_row = rows.tile([1, NQ], F32, tag="u_row",
                                      name='u_row')
                    nc.vector.scalar_tensor_tensor(
                        u_row[:], t_row[:], -1.0 / D, sq_ps[:],
                        ALU.mult, ALU.add)
                    sd_row = rows.tile([1, NQ], F32, tag="sd_row",
                                       name='sd_row')
                    nc.scalar.activation(sd_row[:], u_row[:], AF.Sqrt,
                                         bias=eps_t[:], scale=1.0 / D)
                    a32 = rows.tile([1, NQ], F32, tag="a32", name='a32')
                    nc.vector.reciprocal_approx_fast(a32[:], sd_row[:])
                    nc.vector.tensor_copy(a_bf[0:1, sl], a32[:])
                    b32 = rows.tile([1, NQ], F32, tag="b32", name='b32')
                    nc.vector.scalar_tensor_tensor(
                        b32[:], ss[:], -1.0 / D, a32[:], ALU.mult,
                        ALU.mult)
                    nc.vector.tensor_copy(bstack[0:1, sl], b32[:])

            # mod groups 0-3 (sh_msa, sc_msa) — PE filler while rows run
            for g in range(4):
                mod_group(g, psmod)

            # S1 = 1 + sc_msa (row 0), sh_msa (row 1, cross-partition DMA)
            S1sh1 = rows.tile([2, D], BF16, name='S1sh1')
            nc.vector.tensor_scalar_add(S1sh1[0:1, :], mod_row[0:1, D:2 * D],
                                        1.0)
            nc.scalar.dma_start(S1sh1[1:2, :], mod_row[0:1, 0:D])

            # h = x*A + B  (ch0 first so q can start early)
            with ExitStack() as sec:
                psab = sec.enter_context(
                    tc.tile_pool(name="psab", bufs=2, space="PSUM"))
                for ch in range(2):
                    for k in range(KT):
                        sl = slice(ch * NQ, (ch + 1) * NQ)
                        pa = psab.tile([P, NQ], F32, tag="pA", name='pA')
                        pb = psab.tile([P, NQ], F32, tag="pB", name='pB')
                        nc.tensor.matmul(
                            pa[:], lhsT=S1sh1[0:1, k * P:(k + 1) * P],
                            rhs=a_bf[0:1, sl], start=True, stop=True)
                        nc.tensor.matmul(
                            pb[:], lhsT=S1sh1[:, k * P:(k + 1) * P],
                            rhs=bstack[:, sl], start=True, stop=True)
                        nc.vector.tensor_mul(
                            hT[k][:, sl],
                            xall[:, k * N + ch * NQ:k * N + (ch + 1) * NQ],
                            pa[:])
                        nc.vector.tensor_add(hT[k][:, sl], hT[k][:, sl],
                                             pb[:])

        # phase 2: q, k0/1, v(vg0)
        with ExitStack() as ph:
            wqpool = ph.enter_context(tc.tile_pool(name="p2wq", bufs=1))
            ps = ph.enter_context(
                tc.tile_pool(name="p2ps", bufs=2, space="PSUM"))
            psk = ph.enter_context(
                tc.tile_pool(name="p2psk", bufs=1, space="PSUM"))

            wq_t = wqpool.tile([P, 2 * KT * NQ], BF16, tag="wq", name="wq")
            nc.sync.dma_start(wq_t[:, 0:GW], Wq[:, 0:GW])
            nc.scalar.dma_start(wq_t[:, GW:2 * GW], Wq[:, GW:2 * GW])

            for nt in range(KT):
                vv = vRt[nt].rearrange("p (h w) -> p h w", w=DH + 1)
                nc.gpsimd.memset(vv[:, :, DH:DH + 1], 1.0)

            for g in range(2):
                for dot in range(4):
                    t = 4 * g + dot
                    p = ps.tile([P, NQ], F32, tag="qprj", name='qprj')
                    for k in range(KT):
                        o = g * GW + k * NQ + dot * P
                        nc.tensor.matmul(
                            p[:], lhsT=wq_t[:, o:o + P],
                            rhs=hT[k][:, 0:NQ],
                            start=(k == 0), stop=(k == KT - 1))
                    nc.scalar.activation(qTt[t][:], p[:], AF.Identity,
                                         bias=bqT_s[:, t:t + 1],
                                         scale=DH ** -0.5)

            emit_kT(0, psk)
            emit_kT(1, psk)
            for nt in range(KT):
                emit_v(0, nt, ps)

        # ---------------- phase 3: attention ----------------
        with ExitStack() as ph:
            epool = ph.enter_context(tc.tile_pool(name="p3e", bufs=10))
            spool = ph.enter_context(tc.tile_pool(name="p3s", bufs=1))
            ps_sim = ph.enter_context(
                tc.tile_pool(name="ps_sim", bufs=2, space="PSUM"))
            ps_o = ph.enter_context(
                tc.tile_pool(name="ps_o", bufs=1, space="PSUM"))
            ps_bc = ph.enter_context(
                tc.tile_pool(name="ps_bc", bufs=1, space="PSUM"))
            ps_jit = ph.enter_context(
                tc.tile_pool(name="ps_jit", bufs=1, space="PSUM"))
            ps_modp = ph.enter_context(
                tc.tile_pool(name="ps_modp", bufs=1, space="PSUM"))


            def emit_kT_ch(t, ch, psp):
                g, dot = t // 4, t % 4
                base = g * GW
                sl = slice(ch * NQ, (ch + 1) * NQ)
                pk = psp.tile([P, NQ], F32, tag="vprj", name='kprjn')
                for k in range(KT):
                    o = base + k * NQ + dot * P
                    nc.tensor.matmul(
                        pk[:], lhsT=kg_t[:, o:o + P],
                        rhs=hT[k][:, sl],
                        start=(k == 0), stop=(k == KT - 1))
                nc.vector.tensor_scalar_add(kTt[t][:, sl], pk[:],
                                            bkT[:, t:t + 1])

            for hp in range(H // 2):
                pt = hp
                # JIT work units to weave between sim/exp pairs
                jit = []
                if 0 < hp < 7:
                    jit.append(lambda t=hp + 1: emit_kT_ch(t, 0, ps_jit))
                    jit.append(lambda t=hp + 1: emit_kT_ch(t, 1, ps_jit))
                if hp < 4:
                    jit.append(lambda nt=2 * hp: emit_v(1, nt, ps_jit))
                    jit.append(lambda nt=2 * hp + 1: emit_v(1, nt, ps_jit))
                if 1 <= hp <= 6:
                    jit.append(lambda g=3 + hp: mod_group(g, ps_modp))
                if hp == 6:
                    jit.append(lambda: mod_group(10, ps_modp))
                if hp == 7:
                    jit.append(lambda: mod_group(11, ps_modp))
                if hp == 2:
                    def _gmsa():
                        pg = ps_jit.tile([P, NQ], F32, tag="vprj",
                                         name='gcol')
                        cols_from_row(mod_row[0:1, 2 * D:3 * D], gmsaT, pg)
                    jit.append(_gmsa)
                if hp == 7:
                    def _gmlp():
                        pg = ps_jit.tile([P, NQ], F32, tag="vprj",
                                         name='gcol2')
                        cols_from_row(mod_row[0:1, 5 * D:6 * D], gmlpT, pg)
                    jit.append(_gmlp)

                et = {}
                ji = 0
                for hi in range(2):
                    hh = hi * DH
                    for kp in range(4):
                        psim = ps_sim.tile([P, N], F32, tag="sim",
                                           name='sim')
                        for j in range(2):
                            kt = 2 * kp + j
                            nc.tensor.matmul(
                                psim[:, j * NQ:(j + 1) * NQ],
                                lhsT=kTt[pt][hh:hh + DH,
                                             kt * P:(kt + 1) * P],
                                rhs=qTt[pt][hh:hh + DH, :],
                                start=True, stop=True)
                        e = epool.tile([P, N], BF16, tag="e", name='e')
                        nc.scalar.activation(e[:], psim[:], AF.Exp)
                        et[hi, kp] = e
                        if ji < len(jit):
                            jit[ji]()
                            ji += 1
                while ji < len(jit):
                    jit[ji]()
                    ji += 1
                for hi in range(2):
                    h = 2 * hp + hi
                    hh = hi * DH
                    pos = ps_o.tile([DH + 1, NQ], F32, tag="ov", name='ov')
                    for kp in range(4):
                        for j in range(2):
                            kt = 2 * kp + j
                            nc.tensor.matmul(
                                pos[:],
                                lhsT=vRt[kt][:, h * (DH + 1):
                                             (h + 1) * (DH + 1)],
                                rhs=et[hi, kp][:, j * NQ:(j + 1) * NQ],
                                start=(kt == 0), stop=(kt == KT - 1))
                    # normalize: inv = 1/denom (fast recip), PE broadcast
                    den = spool.tile([1, NQ], F32, tag="den", name='den')
                    nc.vector.tensor_copy(den[:], pos[DH:DH + 1, :])
                    inv32 = spool.tile([1, NQ], F32, tag="inv32",
                                       name='inv32')
                    nc.vector.reciprocal_approx_fast(inv32[:], den[:])
                    inv16 = spool.tile([1, NQ], BF16, tag="inv16",
                                       name='inv16')
                    nc.vector.tensor_copy(inv16[:], inv32[:])
                    pb = ps_bc.tile([DH, NQ], F32, tag="bc", name='bc')
                    nc.tensor.matmul(pb[:], lhsT=ones_row[0:1, 0:DH],
                                     rhs=inv16[:], start=True, stop=True)
                    binv = spool.tile([DH, NQ], F32, tag="binv",
                                      name='binv')
                    nc.vector.tensor_copy(binv[:], pb[:])
                    nc.vector.tensor_mul(outT[pt][hh:hh + DH, :],
                                         pos[0:DH, :], binv[:])

        wkv_cm.__exit__(None, None, None)
        qkv_cm.__exit__(None, None, None)
        hT_cm.__exit__(None, None, None)
        wada_cm.__exit__(None, None, None)

        # ---------------- phase 4: Wo + residual + ln2 ----------------
        wpool = root.enter_context(tc.tile_pool(name="p5w", bufs=2,
                                                side='right'))
        wopool_r = root.enter_context(tc.tile_pool(name="p4wo", bufs=1,
                                                   side='right'))
        wo_t = wopool_r.tile([P, 2 * KT * NQ], BF16, tag="wo", name='wo')
        nc.sync.dma_start(wo_t[:, 0:GW], Wo[:, 0:GW])
        nc.sync.dma_start(wo_t[:, GW:2 * GW], Wo[:, GW:2 * GW])
        w1pair_tiles = {}
        w1_p0 = wpool.tile([P, 2 * KT * NQ], BF16, tag="w1", name='w1')
        nc.gpsimd.dma_start(w1_p0[:, 0:GW], W1[:, 0:GW])
        nc.sync.dma_start(w1_p0[:, GW:2 * GW], W1[:, GW:2 * GW])
        w1pair_tiles[0] = w1_p0
        x1p = root.enter_context(tc.tile_pool(name="x1p", bufs=1,
                                              side='right'))
        x1t = [x1p.tile([P, NQ], BF16, tag=f"x1{k}", name=f"x1{k}")
               for k in range(KT)]
        h2p = root.enter_context(tc.tile_pool(name="h2p", bufs=1,
                                              side='right'))
        h2t = [h2p.tile([P, NQ], BF16, tag=f"h2{k}", name=f"h2{k}")
               for k in range(KT)]
        x1bp = root.enter_context(tc.tile_pool(name="x1bp", bufs=1,
                                               side='right'))
        x1b = [x1bp.tile([P, NQ], BF16, tag=f"x1b{k}", name=f"x1b{k}")
               for k in range(KT)]

        with ExitStack() as ph:
            rows4 = ph.enter_context(tc.tile_pool(name="p4rows", bufs=1))
            psy = ph.enter_context(
                tc.tile_pool(name="psy", bufs=2, space="PSUM"))

            # xo2[t] = x_own + gmsa*bo_eff  (per-partition col broadcast)
            bo2 = rows4.tile([P, KT], F32, name='bo2')
            nc.vector.tensor_mul(bo2[:], boT[:], gmsaT[:])
            xo2 = [rows4.tile([P, NQ], BF16, tag=f"xo2{k}", name=f"xo2{k}")
                   for k in range(KT)]
            for k in range(KT):
                nc.scalar.activation(xo2[k][:], xall[:, k * N:k * N + NQ],
                                     AF.Identity, bias=bo2[:, k:k + 1])

            psstat2 = ph.enter_context(
                tc.tile_pool(name="psstat2", bufs=1, space="PSUM"))
            sq2pool = ph.enter_context(tc.tile_pool(name="p4sq", bufs=4))
            ss2 = psstat2.tile([1, NQ], F32, tag="st2s", name='st2s')
            sq2_ps = psstat2.tile([1, NQ], F32, tag="st2q", name='st2q')
            for g in range(2):
                for dot in range(4):
                    t = 4 * g + dot
                    p = psy.tile([P, NQ], F32, tag="y1", name='y1')
                    for k in range(KT):
                        o = g * GW + k * NQ + dot * P
                        nc.tensor.matmul(
                            p[:], lhsT=wo_t[:, o:o + P],
                            rhs=outT[k][:],
                            start=(k == 0), stop=(k == KT - 1))
                    nc.vector.scalar_tensor_tensor(
                        x1t[t][:], p[:], gmsaT[:, t:t + 1], xo2[t][:],
                        ALU.mult, ALU.add)
                    sq = sq2pool.tile([P, NQ], BF16, tag="x1sq",
                                      name='x1sq')
                    nc.scalar.activation(sq[:], x1t[t][:], AF.Square)
                    nc.tensor.matmul(ss2[:], lhsT=ones_col[:],
                                     rhs=x1t[t][:],
                                     start=(t == 0), stop=(t == KT - 1))
                    nc.tensor.matmul(sq2_ps[:], lhsT=ones_col[:],
                                     rhs=sq[:],
                                     start=(t == 0), stop=(t == KT - 1))

            # ln2 rows (stats accumulated inside the Wo loop above)
            a2_bf = rows4.tile([1, NQ], BF16, name='a2_bf')
            b2stack = rows4.tile([2, NQ], BF16, name='b2stack')
            nc.gpsimd.memset(b2stack[:, :], 1.0)  # row1 stays ones
            t_row = rows4.tile([1, NQ], F32, name='t2_row')
            nc.scalar.activation(t_row[:], ss2[:], AF.Square)
            u_row = rows4.tile([1, NQ], F32, name='u2_row')
            nc.vector.scalar_tensor_tensor(
                u_row[:], t_row[:], -1.0 / D, sq2_ps[:], ALU.mult,
                ALU.add)
            sd_row = rows4.tile([1, NQ], F32, name='sd2_row')
            nc.scalar.activation(sd_row[:], u_row[:], AF.Sqrt,
                                 bias=eps_t[:], scale=1.0 / D)
            a32 = rows4.tile([1, NQ], F32, name='a232')
            nc.vector.reciprocal_approx_fast(a32[:], sd_row[:])
            nc.vector.tensor_copy(a2_bf[:], a32[:])
            b32 = rows4.tile([1, NQ], F32, name='b232')
            nc.vector.scalar_tensor_tensor(
                b32[:], ss2[:], -1.0 / D, a32[:], ALU.mult, ALU.mult)
            nc.vector.tensor_copy(b2stack[0:1, :], b32[:])

            S2sh2 = rows4.tile([2, D], BF16, name='S2sh2')
            nc.vector.tensor_scalar_add(S2sh2[0:1, :],
                                        mod_row[0:1, 4 * D:5 * D], 1.0)
            nc.scalar.dma_start(S2sh2[1:2, :], mod_row[0:1, 3 * D:4 * D])

            # x1b[t] = x1 + gmlp*b2  (for the MLP epilogue)
            b2g = rows4.tile([P, KT], F32, name='b2g')
            nc.vector.tensor_mul(b2g[:], b2T[:], gmlpT[:])
            for k in range(KT):
                nc.scalar.activation(x1b[k][:], x1t[k][:], AF.Identity,
                                     bias=b2g[:, k:k + 1])

            with ExitStack() as sec:
                psab2 = sec.enter_context(
                    tc.tile_pool(name="psab2", bufs=2, space="PSUM"))
                for k in range(KT):
                    pa = psab2.tile([P, NQ], F32, tag="pA2", name='pA2')
                    pb = psab2.tile([P, NQ], F32, tag="pB2", name='pB2')
                    nc.tensor.matmul(
                        pa[:], lhsT=S2sh2[0:1, k * P:(k + 1) * P],
                        rhs=a2_bf[:], start=True, stop=True)
                    nc.tensor.matmul(
                        pb[:], lhsT=S2sh2[:, k * P:(k + 1) * P],
                        rhs=b2stack[:], start=True, stop=True)
                    nc.vector.tensor_mul(h2t[k][:], x1t[k][:], pa[:])
                    nc.vector.tensor_add(h2t[k][:], h2t[k][:], pb[:])

        op_cm.__exit__(None, None, None)
        xp_cm.__exit__(None, None, None)

        # ---------------- phase 5: MLP ----------------
        with ExitStack() as ph:
            gp = ph.enter_context(tc.tile_pool(name="gp", bufs=1))
            gTt = [gp.tile([P, NQ], BF16, tag=f"g{m}", name=f"g{m}")
                   for m in range(MT)]
            w2pool = ph.enter_context(tc.tile_pool(name="p5w2", bufs=2))
            opool = ph.enter_context(tc.tile_pool(name="p5o", bufs=3))
            ps1 = ph.enter_context(
                tc.tile_pool(name="ps1", bufs=3, space="PSUM"))
            ps2 = ph.enter_context(
                tc.tile_pool(name="ps2", bufs=1, space="PSUM"))

            for gp2 in range(4):   # 4 pair loads of 2 groups
                if gp2 in w1pair_tiles:
                    w1_g = w1pair_tiles[gp2]
                else:
                    w1_g = wpool.tile([P, 2 * KT * NQ], BF16, tag="w1",
                                      name='w1')
                    eng = nc.gpsimd if gp2 % 2 else nc.sync
                    eng2 = nc.sync if gp2 % 2 else nc.gpsimd
                    eng.dma_start(w1_g[:, 0:GW],
                                  W1[:, 2 * gp2 * GW:(2 * gp2 + 1) * GW])
                    eng2.dma_start(w1_g[:, GW:2 * GW],
                                   W1[:, (2 * gp2 + 1) * GW:
                                      (2 * gp2 + 2) * GW])
                for gh in range(2):
                    g = 2 * gp2 + gh
                    for dot in range(4):
                        m = 4 * g + dot
                        p = ps1.tile([P, NQ], F32, tag="m1", name='m1')
                        for k in range(KT):
                            o = gh * GW + k * NQ + dot * P
                            nc.tensor.matmul(
                                p[:], lhsT=w1_g[:, o:o + P],
                                rhs=h2t[k][:],
                                start=(k == 0), stop=(k == KT - 1))
                        nc.scalar.activation(gTt[m][:], p[:],
                                             AF.Gelu_apprx_tanh,
                                             bias=b1T[:, m:m + 1])

            for half in range(2):
                pacc = [ps2.tile([P, NQ], F32, tag=f"acc{d}",
                                 name=f"acc{d}") for d in range(4)]
                for mg2 in range(2):   # 2 pair loads of 16 k-chunks
                    w2c = w2pool.tile([P, 2 * KT * NQ], BF16, tag="w2",
                                      name='w2')
                    blk = half * 4 + 2 * mg2
                    eng = nc.gpsimd if (half + mg2) % 2 else nc.sync
                    eng2 = nc.sync if (half + mg2) % 2 else nc.gpsimd
                    eng.dma_start(w2c[:, 0:GW],
                                  W2[:, blk * GW:(blk + 1) * GW])
                    eng2.dma_start(w2c[:, GW:2 * GW],
                                   W2[:, (blk + 1) * GW:(blk + 2) * GW])
                    for kk in range(2 * KT):
                        mk = 2 * mg2 * KT + kk
                        for d in range(4):
                            o = kk * NQ + d * P
                            nc.tensor.matmul(
                                pacc[d][:], lhsT=w2c[:, o:o + P],
                                rhs=gTt[mk][:],
                                start=(mk == 0), stop=(mk == MT - 1))
                for d in range(4):
                    t = half * 4 + d
                    yt = opool.tile([P, NQ], F32, tag="yout", name='yout')
                    nc.vector.scalar_tensor_tensor(
                        yt[:], pacc[d][:], gmlpT[:, t:t + 1], x1b[t][:],
                        ALU.mult, ALU.add)
                    nc.sync.dma_start(yT[t * P:(t + 1) * P, :], yt[:])

    nc.compile()
    return nc


_NC = None


def _get_nc():
    global _NC
    if _NC is None:
        _NC = build()
    return _NC


def _pack_cols(W, bf):
    """[Din, C] -> [P, (C//512)*Din*... ] device-tile-contiguous layout."""
    Din, C = W.shape
    kt = Din // P
    blocks = [W.reshape(kt, P, C)[:, :, g * 512:(g + 1) * 512]
              .transpose(1, 0, 2).reshape(P, kt * 512)
              for g in range(C // 512)]
    return np.ascontiguousarray(np.concatenate(blocks, axis=1)).astype(bf)


def _prep_inputs(x, c, Wq, bq, Wkv, bkv, Wo, bo, W1, b1, W2, b2, Wada, bada):
    import ml_dtypes
    f = np.float32
    bf = ml_dtypes.bfloat16
    col = lambda v, n: np.ascontiguousarray(
        np.asarray(v, f).reshape(n, P).T)
    bkv = np.asarray(bkv, f)
    Wo_f = np.asarray(Wo, f)
    bo_eff = np.asarray(bo, f) + bkv[D:] @ Wo_f   # fold v-bias through Wo
    W2f = np.asarray(W2, f)
    w2blocks = [W2f.reshape(MT, P, D)[mg * 8:(mg + 1) * 8, :,
                                      half * 512:(half + 1) * 512]
                .transpose(1, 0, 2).reshape(P, KT * 512)
                for half in range(2) for mg in range(4)]
    shared = {
        "Wq": _pack_cols(np.asarray(Wq, f), bf),
        "Wkv": _pack_cols(np.asarray(Wkv, f), bf),
        "Wo": _pack_cols(Wo_f, bf),
        "W1": _pack_cols(np.asarray(W1, f), bf),
        "W2": np.ascontiguousarray(
            np.concatenate(w2blocks, axis=1)).astype(bf),
        "Wada": _pack_cols(np.asarray(Wada, f), bf),
        "bada_r": np.asarray(bada, f).reshape(1, -1).astype(bf),
        "bq_c": col(np.asarray(bq, f) * (DH ** -0.5), KT),
        "bk_c": col(bkv[:D], KT),
        "bo_c": col(bo_eff, KT),
        "b1_c": col(b1, MT), "b2_c": col(b2, KT),
    }
    in_maps = []
    for core in range(NCORES):
        b, half = core // 2, core % 2
        xb = np.asarray(x[b], f)
        perm = np.concatenate(
            [xb[half * NQ:(half + 1) * NQ],
             xb[(1 - half) * NQ:(2 - half) * NQ]], axis=0)
        m = dict(shared)
        px = perm.T  # [D, N] feature-major
        m["xT"] = np.ascontiguousarray(
            np.concatenate([px[k * P:(k + 1) * P, :] for k in range(KT)],
                           axis=1)).astype(bf)
        m["crow"] = np.asarray(c[b:b + 1], f).astype(bf)
        in_maps.append(m)
    return in_maps


def _run(inputs, trace=False):
    nc = _get_nc()
    in_maps = _prep_inputs(**inputs)
    res = run_bass_kernel_spmd(nc, in_maps, core_ids=list(range(NCORES)),
                               trace=trace)
    B = 4
    y = np.empty((B, N, D), np.float32)
    for core in range(NCORES):
        b, half = core // 2, core % 2
        y[b, half * NQ:(half + 1) * NQ, :] = res.results[core]["yT"].T
    return y, res


def kernel(**inputs):
    y, _ = _run(inputs, trace=False)
    return y
